# revision 3
# baseline (speedup 1.0000x reference)
"""DualReconstruction Trainium2 kernel (8 NeuronCores, data-parallel over B).

Math (per reference):
  qn = l2norm(query), sn = l2norm(support)
  sim_q[b,k,i,j] = <qn[b,i], sn[k,j]>;  attn_q = softmax_j(sim_q)
  recon_q[b,i,c] = sum_{k,j} attn_q[b,k,i,j] support[k,j,c]
  sim_s = sim_q transposed;              attn_s = softmax over query dim
  recon_s[k,i,c] = sum_{b,j} attn_s[k,b,i,j] query[b,j,c]   (all-reduced over b)
  cosine losses over reconstructions.

Sharding: each of the 8 cores owns 4 of the 32 queries (B dim) and the full
support set.  Only recon_s (+ the q_loss scalar) needs an AllReduce.

|sim| <= 1 (unit vectors), so softmax skips the max-subtraction: exp is
computed directly and the row-sum comes for free via activation accum_out.
Matmuls run in float32r (e8m11) at full PE speed.
"""

import sys

sys.path.insert(0, "/opt/trn_rl_repo")

import numpy as np

import concourse.bacc as bacc
import concourse.mybir as mybir
import concourse.tile as tile
from concourse.bass_utils import run_bass_kernel_spmd
from concourse.masks import make_identity

F32 = mybir.dt.float32
F32R = mybir.dt.float32r
AF = mybir.ActivationFunctionType
AX = mybir.AxisListType.X

B, L, C, K = 32, 196, 384, 25
NCORES = 8
BL = B // NCORES          # 4 queries per core
QR = BL * L               # 784 stacked query rows
SR = K * L                # 4900 stacked support rows
G = 5                     # support images per group
NG = K // G               # 5 groups
GL = G * L                # 980 support rows per group
QCH = 7                   # ceil(784/128)
GCH = 8                   # ceil(980/128)
CCH = 3                   # 384/128
EPS = 1e-8

CNT_Q = [128] * 6 + [16]
CNT_G = [128] * 7 + [84]

# exp/rowsum sub-slices of the two 490-wide S1 PSUM passes:
# (col0, col1, accum_idx); k2 spans both passes (idx 2 + idx 3).
_S1_SLICES = {
    (0, 490): [(0, 196, 0), (196, 392, 1), (392, 490, 2)],
    (490, 980): [(490, 588, 3), (588, 784, 4), (784, 980, 5)],
}


def _pieces(lo, hi):
    """Intersect row range [lo,hi) with 128-row chunks -> (chunk, a, b)."""
    out = []
    for ch in range(lo // 128, (hi + 127) // 128):
        a = max(lo - 128 * ch, 0)
        b = min(hi - 128 * ch, 128)
        if a < b:
            out.append((ch, a, b))
    return out


def _build():
    nc = bacc.Bacc()
    q_in = nc.declare_dram_parameter("q_in", [QR, C], F32, isOutput=False)
    s_in = nc.declare_dram_parameter("s_in", [SR, C], F32, isOutput=False)
    # row->slab assignment matrices (host-built constants):
    # asn_y[k, r] = 1 if group row r belongs to support image k (r//196 == k)
    # asn_x[b, r] = 1 if query row r belongs to image b
    asn_y_in = nc.declare_dram_parameter("asn_y", [G, GCH * 128], F32, isOutput=False)
    asn_x_in = nc.declare_dram_parameter("asn_x", [BL, QCH * 128], F32, isOutput=False)
    attn_q_o = nc.declare_dram_parameter("attn_q", [BL, K, L, L], F32, isOutput=True)
    attn_s_o = nc.declare_dram_parameter("attn_s", [K, BL, L, L], F32, isOutput=True)
    recon_q_o = nc.declare_dram_parameter("recon_q", [QR, C], F32, isOutput=True)
    recon_s_o = nc.declare_dram_parameter("recon_s", [SR, C], F32, isOutput=True)
    loss_o = nc.declare_dram_parameter("loss", [1, 3], F32, isOutput=True)

    # collective staging: rows 0..SR-1 = partial recon_s, row SR col 0 = q_loss partial
    cc_in = nc.dram_tensor("cc_in", [SR + 1, C], F32)
    cc_out = nc.dram_tensor("cc_out", [SR + 1, C], F32, addr_space="Shared")

    with (
        tile.TileContext(nc) as tc,
        tc.tile_pool(name="pp", bufs=1) as pp,
        tc.tile_pool(name="gp", bufs=1) as gp,
        tc.tile_pool(name="scr", bufs=2) as scr,
        tc.tile_pool(name="ps_sim", bufs=4, space="PSUM") as ps_sim,
        tc.tile_pool(name="ps_bc", bufs=1, space="PSUM") as ps_bc,
        tc.tile_pool(name="ps_rc", bufs=2, space="PSUM") as ps_rc,
    ):
        # ---------- constants ----------
        ident = pp.tile([128, 128], F32)
        make_identity(nc, ident)
        ones_f = pp.tile([128, 1], F32)  # all-ones column (partition sums)
        nc.vector.memset(ones_f, 1.0)
        # row->slab assignment matrices (from host), used to build
        # per-chunk broadcast matrices on the PE
        asn_y = pp.tile([G, GCH * 128], F32R)
        nc.sync.dma_start(out=asn_y, in_=asn_y_in.ap().bitcast(F32R))
        asn_x = pp.tile([BL, QCH * 128], F32R)
        nc.sync.dma_start(out=asn_x, in_=asn_x_in.ap().bitcast(F32R))

        # ---------- load queries, normalize, transpose ----------
        q_r = pp.tile([128, QCH, C], F32R)
        for ch in range(QCH):
            cnt = CNT_Q[ch]
            nc.sync.dma_start(
                out=q_r[:cnt, ch, :],
                in_=q_in.ap()[128 * ch : 128 * ch + cnt, :].bitcast(F32R),
            )
        qss = pp.tile([128, QCH], F32)
        nc.vector.memset(qss, 1.0)
        for ch in range(QCH):
            cnt = CNT_Q[ch]
            sq = scr.tile([128, C], F32, tag="sq")
            nc.scalar.activation(
                sq[:cnt], q_r[:cnt, ch, :].bitcast(F32), AF.Square,
                accum_out=qss[:cnt, ch : ch + 1],
            )
        qnorm = pp.tile([128, QCH], F32)
        nc.scalar.activation(qnorm, qss, AF.Sqrt)
        nc.vector.tensor_scalar_max(qnorm, qnorm, EPS)
        inv_nq = pp.tile([128, QCH], F32)
        nc.vector.reciprocal(inv_nq, qnorm)
        qn = pp.tile([128, QCH, C], F32)
        for ch in range(QCH):
            cnt = CNT_Q[ch]
            nc.vector.tensor_scalar_mul(
                qn[:cnt, ch, :], q_r[:cnt, ch, :].bitcast(F32), inv_nq[:cnt, ch : ch + 1]
            )
        qnT = pp.tile([128, CCH, QR], F32R)
        for ch in range(QCH):
            cnt = CNT_Q[ch]
            for cc in range(CCH):
                ptr = ps_sim.tile([128, 490], F32, tag="sim")
                nc.tensor.transpose(
                    ptr[:128, :cnt], qn[:cnt, ch, 128 * cc : 128 * (cc + 1)],
                    ident[:cnt, :cnt],
                )
                nc.vector.tensor_copy(
                    qnT[:, cc, 128 * ch : 128 * ch + cnt], ptr[:128, :cnt].bitcast(F32R)
                )

        # ---------- support norms (group-chunk layout) ----------
        sss = pp.tile([128, NG * GCH], F32)
        nc.vector.memset(sss, 1.0)
        for g in range(NG):
            for j in range(GCH):
                cnt = CNT_G[j]
                r0 = GL * g + 128 * j
                st = scr.tile([128, C], F32, tag="schunk")
                nc.sync.dma_start(out=st[:cnt], in_=s_in.ap()[r0 : r0 + cnt, :])
                sq = scr.tile([128, C], F32, tag="sq")
                gc = g * GCH + j
                nc.scalar.activation(
                    sq[:cnt], st[:cnt], AF.Square, accum_out=sss[:cnt, gc : gc + 1]
                )
        snorm = pp.tile([128, NG * GCH], F32)
        nc.scalar.activation(snorm, sss, AF.Sqrt)
        nc.vector.tensor_scalar_max(snorm, snorm, EPS)
        inv_ns = pp.tile([128, NG * GCH], F32)
        nc.vector.reciprocal(inv_ns, snorm)

        rq_acc = pp.tile([128, QCH, C], F32)
        nc.vector.memset(rq_acc, 0.0)

        # ---------- main loop over support groups ----------
        for g in range(NG):
            k0 = G * g
            s_r = gp.tile([128, GCH, C], F32R, tag="s_r")
            for j in range(GCH):
                cnt = CNT_G[j]
                r0 = GL * g + 128 * j
                nc.sync.dma_start(
                    out=s_r[:cnt, j, :],
                    in_=s_in.ap()[r0 : r0 + cnt, :].bitcast(F32R),
                )
            # normalized-support transpose for this group
            snT = gp.tile([128, CCH, GL], F32R, tag="snT")
            for j in range(GCH):
                cnt = CNT_G[j]
                gc = g * GCH + j
                snc = scr.tile([128, C], F32, tag="snc")
                nc.vector.tensor_scalar_mul(
                    snc[:cnt], s_r[:cnt, j, :].bitcast(F32), inv_ns[:cnt, gc : gc + 1]
                )
                for cc in range(CCH):
                    ptr = ps_sim.tile([128, 490], F32, tag="sim")
                    nc.tensor.transpose(
                        ptr[:128, :cnt], snc[:cnt, 128 * cc : 128 * (cc + 1)],
                        ident[:cnt, :cnt],
                    )
                    nc.vector.tensor_copy(
                        snT[:, cc, 128 * j : 128 * j + cnt], ptr[:128, :cnt].bitcast(F32R)
                    )

            # S1 = qn @ snT (rows: stacked queries), exp + per-k rowsums
            E1 = gp.tile([128, QCH, GL], F32, tag="E1")
            rsq_p = gp.tile([128, QCH, 6], F32, tag="rsq_p")
            nc.vector.memset(rsq_p, 1.0)
            for ch in range(QCH):
                cnt = CNT_Q[ch]
                for (n0, n1) in ((0, 490), (490, 980)):
                    pm = ps_sim.tile([128, 490], F32, tag="sim")
                    for cc in range(CCH):
                        nc.tensor.matmul(
                            pm[:cnt, : n1 - n0],
                            qnT[:, cc, 128 * ch : 128 * ch + cnt],
                            snT[:, cc, n0:n1],
                            start=(cc == 0), stop=(cc == CCH - 1),
                        )
                    for (a, b2, idx) in _S1_SLICES[(n0, n1)]:
                        nc.scalar.activation(
                            E1[:cnt, ch, a:b2], pm[:cnt, a - n0 : b2 - n0], AF.Exp,
                            accum_out=rsq_p[:cnt, ch, idx : idx + 1],
                        )
            rsq_inv = gp.tile([128, QCH, G], F32, tag="rsq_inv")
            nc.vector.tensor_copy(rsq_inv[:, :, 0], rsq_p[:, :, 0])
            nc.vector.tensor_copy(rsq_inv[:, :, 1], rsq_p[:, :, 1])
            nc.vector.tensor_add(rsq_inv[:, :, 2], rsq_p[:, :, 2], rsq_p[:, :, 3])
            nc.vector.tensor_copy(rsq_inv[:, :, 3], rsq_p[:, :, 4])
            nc.vector.tensor_copy(rsq_inv[:, :, 4], rsq_p[:, :, 5])
            nc.vector.reciprocal(rsq_inv, rsq_inv)

            # S2 = sn @ qnT (rows: this group's support), exp + per-b rowsums
            E2 = gp.tile([128, GCH, QR], F32, tag="E2")
            rss_s = gp.tile([128, GCH, BL], F32, tag="rss_s")
            nc.vector.memset(rss_s, 1.0)
            for j in range(GCH):
                cnt = CNT_G[j]
                for (n0, n1) in ((0, 392), (392, 784)):
                    pm = ps_sim.tile([128, 490], F32, tag="sim")
                    for cc in range(CCH):
                        nc.tensor.matmul(
                            pm[:cnt, : n1 - n0],
                            snT[:, cc, 128 * j : 128 * j + cnt],
                            qnT[:, cc, n0:n1],
                            start=(cc == 0), stop=(cc == CCH - 1),
                        )
                    for bi in range(2):
                        b = n0 // 196 + bi
                        a0 = 196 * b
                        nc.scalar.activation(
                            E2[:cnt, j, a0 : a0 + 196], pm[:cnt, a0 - n0 : a0 - n0 + 196],
                            AF.Exp, accum_out=rss_s[:cnt, j, b : b + 1],
                        )
            rss_inv = gp.tile([128, GCH, BL], F32, tag="rss_inv")
            nc.vector.reciprocal(rss_inv, rss_s)

            # transpose the inverse rowsums into rows (for broadcasts)
            irqT = gp.tile([G, QR], F32R, tag="irqT")
            for ch in range(QCH):
                cnt = CNT_Q[ch]
                ptr = ps_sim.tile([128, 490], F32, tag="sim")
                nc.tensor.transpose(
                    ptr[:G, :cnt], rsq_inv[:cnt, ch, :], ident[:cnt, :cnt]
                )
                nc.vector.tensor_copy(
                    irqT[:, 128 * ch : 128 * ch + cnt], ptr[:G, :cnt].bitcast(F32R)
                )
            irsT = gp.tile([BL, GL], F32R, tag="irsT")
            for j in range(GCH):
                cnt = CNT_G[j]
                ptr = ps_sim.tile([128, 490], F32, tag="sim")
                nc.tensor.transpose(
                    ptr[:BL, :cnt], rss_inv[:cnt, j, :], ident[:cnt, :cnt]
                )
                nc.vector.tensor_copy(
                    irsT[:, 128 * j : 128 * j + cnt], ptr[:BL, :cnt].bitcast(F32R)
                )

            # Y = attn_q^T = E2 * inv_rsq[bq] (row broadcast via asn matmul)
            Y = gp.tile([128, GCH, QR], F32R, tag="Y")
            for j in range(GCH):
                cnt = CNT_G[j]
                pb = ps_bc.tile([128, 2, 512], F32, tag="bc")
                for h in range(2):
                    nc.tensor.matmul(
                        pb[:, h, :392],
                        asn_y[:, 128 * j : 128 * (j + 1)],
                        irqT[:, 392 * h : 392 * (h + 1)],
                        start=True, stop=True,
                    )
                    nc.vector.tensor_mul(
                        Y[:cnt, j, 392 * h : 392 * (h + 1)],
                        E2[:cnt, j, 392 * h : 392 * (h + 1)].bitcast(F32R),
                        pb[:cnt, h, :392].bitcast(F32R),
                    )
            # X = attn_s^T = E1 * inv_rss[ks] (row broadcast via asn matmul)
            X = gp.tile([128, QCH, GL], F32R, tag="X")
            for ch in range(QCH):
                cnt = CNT_Q[ch]
                pb = ps_bc.tile([128, 2, 512], F32, tag="bc")
                for h in range(2):
                    nc.tensor.matmul(
                        pb[:, h, :490],
                        asn_x[:, 128 * ch : 128 * (ch + 1)],
                        irsT[:, 490 * h : 490 * (h + 1)],
                        start=True, stop=True,
                    )
                    nc.vector.tensor_mul(
                        X[:cnt, ch, 490 * h : 490 * (h + 1)],
                        E1[:cnt, ch, 490 * h : 490 * (h + 1)].bitcast(F32R),
                        pb[:cnt, h, :490].bitcast(F32R),
                    )

            # attn_q: scale E1 rows in place, then DMA out
            for ch in range(QCH):
                cnt = CNT_Q[ch]
                for kl in range(G):
                    nc.vector.tensor_scalar_mul(
                        E1[:cnt, ch, 196 * kl : 196 * (kl + 1)],
                        E1[:cnt, ch, 196 * kl : 196 * (kl + 1)],
                        rsq_inv[:cnt, ch, kl : kl + 1],
                    )
            for b in range(BL):
                for kl in range(G):
                    for (ch, a, b2) in _pieces(196 * b, 196 * (b + 1)):
                        i0 = 128 * ch + a - 196 * b
                        nc.sync.dma_start(
                            out=attn_q_o.ap()[b, k0 + kl, i0 : i0 + (b2 - a), :],
                            in_=E1[a:b2, ch, 196 * kl : 196 * (kl + 1)],
                        )
            # attn_s: scale E2 rows in place, then DMA out
            for j in range(GCH):
                cnt = CNT_G[j]
                for b in range(BL):
                    nc.vector.tensor_scalar_mul(
                        E2[:cnt, j, 196 * b : 196 * (b + 1)],
                        E2[:cnt, j, 196 * b : 196 * (b + 1)],
                        rss_inv[:cnt, j, b : b + 1],
                    )
            for kl in range(G):
                for b in range(BL):
                    for (j, a, b2) in _pieces(196 * kl, 196 * (kl + 1)):
                        i0 = 128 * j + a - 196 * kl
                        nc.sync.dma_start(
                            out=attn_s_o.ap()[k0 + kl, b, i0 : i0 + (b2 - a), :],
                            in_=E2[a:b2, j, 196 * b : 196 * (b + 1)],
                        )

            # recon_q partial: accumulate Y^T @ support over this group
            for ch in range(QCH):
                cnt = CNT_Q[ch]
                pr = ps_rc.tile([128, C], F32, tag="rc")
                for j in range(GCH):
                    cj = CNT_G[j]
                    nc.tensor.matmul(
                        pr[:cnt, :],
                        Y[:cj, j, 128 * ch : 128 * ch + cnt],
                        s_r[:cj, j, :],
                        start=(j == 0), stop=(j == GCH - 1),
                    )
                nc.vector.tensor_add(rq_acc[:cnt, ch, :], rq_acc[:cnt, ch, :], pr[:cnt, :])

            # recon_s rows (final for this group, pre-allreduce)
            for j in range(GCH):
                cnt = CNT_G[j]
                pr = ps_rc.tile([128, C], F32, tag="rc")
                for ch in range(QCH):
                    cq = CNT_Q[ch]
                    nc.tensor.matmul(
                        pr[:cnt, :],
                        X[:cq, ch, 128 * j : 128 * j + cnt],
                        q_r[:cq, ch, :],
                        start=(ch == 0), stop=(ch == QCH - 1),
                    )
                stg = scr.tile([128, C], F32, tag="rs_stage")
                nc.scalar.copy(stg[:cnt], pr[:cnt, :])
                r0 = GL * g + 128 * j
                nc.sync.dma_start(out=cc_in.ap()[r0 : r0 + cnt, :], in_=stg[:cnt])

        # ---------- epilogue: recon_q out + q_loss partial ----------
        rqss = pp.tile([128, QCH], F32)
        nc.vector.memset(rqss, 1.0)
        dots_q = pp.tile([128, QCH], F32)
        nc.vector.memset(dots_q, 0.0)
        for ch in range(QCH):
            cnt = CNT_Q[ch]
            nc.sync.dma_start(
                out=recon_q_o.ap()[128 * ch : 128 * ch + cnt, :], in_=rq_acc[:cnt, ch, :]
            )
            sq = scr.tile([128, C], F32, tag="sq")
            nc.scalar.activation(
                sq[:cnt], rq_acc[:cnt, ch, :], AF.Square, accum_out=rqss[:cnt, ch : ch + 1]
            )
            t = scr.tile([128, C], F32, tag="dot")
            nc.vector.tensor_mul(t[:cnt], rq_acc[:cnt, ch, :], qn[:cnt, ch, :])
            nc.vector.reduce_sum(dots_q[:cnt, ch : ch + 1], t[:cnt], axis=AX)
        rqn = pp.tile([128, QCH], F32)
        nc.scalar.activation(rqn, rqss, AF.Sqrt)
        nc.vector.tensor_scalar_max(rqn, rqn, EPS)
        inv_rqn = pp.tile([128, QCH], F32)
        nc.vector.reciprocal(inv_rqn, rqn)
        nc.vector.tensor_mul(dots_q, dots_q, inv_rqn)
        dcs = pp.tile([128, 1], F32)
        nc.vector.reduce_sum(dcs, dots_q, axis=AX)
        pq = ps_sim.tile([128, 490], F32, tag="sim")
        nc.tensor.matmul(pq[:1, :1], ones_f[:, :1], dcs[:, :1], start=True, stop=True)
        zrow = pp.tile([1, C], F32)
        nc.vector.memset(zrow, 0.0)
        nc.scalar.copy(zrow[0:1, 0:1], pq[:1, :1])
        nc.sync.dma_start(out=cc_in.ap()[SR : SR + 1, :], in_=zrow)

        # ---------- allreduce recon_s partials + q_loss partial ----------
        nc.gpsimd.collective_compute(
            "AllReduce",
            mybir.AluOpType.add,
            replica_groups=[list(range(NCORES))],
            ins=[cc_in.ap()],
            outs=[cc_out.ap()],
        )

        # ---------- final: recon_s out + s_loss + losses ----------
        dots_s = pp.tile([128, NG * GCH], F32)
        nc.vector.memset(dots_s, 0.0)
        rsss = pp.tile([128, NG * GCH], F32)
        nc.vector.memset(rsss, 1.0)
        for g in range(NG):
            for j in range(GCH):
                cnt = CNT_G[j]
                gc = g * GCH + j
                r0 = GL * g + 128 * j
                rsg = scr.tile([128, C], F32, tag="rsg")
                nc.sync.dma_start(out=rsg[:cnt], in_=cc_out.ap()[r0 : r0 + cnt, :])
                nc.sync.dma_start(out=recon_s_o.ap()[r0 : r0 + cnt, :], in_=rsg[:cnt])
                sq = scr.tile([128, C], F32, tag="sq")
                nc.scalar.activation(
                    sq[:cnt], rsg[:cnt], AF.Square, accum_out=rsss[:cnt, gc : gc + 1]
                )
                st = scr.tile([128, C], F32, tag="schunk")
                nc.sync.dma_start(out=st[:cnt], in_=s_in.ap()[r0 : r0 + cnt, :])
                t = scr.tile([128, C], F32, tag="dot")
                nc.vector.tensor_mul(t[:cnt], rsg[:cnt], st[:cnt])
                nc.vector.reduce_sum(dots_s[:cnt, gc : gc + 1], t[:cnt], axis=AX)
        rsn = pp.tile([128, NG * GCH], F32)
        nc.scalar.activation(rsn, rsss, AF.Sqrt)
        nc.vector.tensor_scalar_max(rsn, rsn, EPS)
        inv_rsn = pp.tile([128, NG * GCH], F32)
        nc.vector.reciprocal(inv_rsn, rsn)
        nc.vector.tensor_mul(dots_s, dots_s, inv_rsn)
        nc.vector.tensor_mul(dots_s, dots_s, inv_ns)
        scs = pp.tile([128, 1], F32)
        nc.vector.reduce_sum(scs, dots_s, axis=AX)
        psm = ps_sim.tile([128, 490], F32, tag="sim")
        nc.tensor.matmul(psm[:1, :1], ones_f[:, :1], scs[:, :1], start=True, stop=True)

        qg = scr.tile([1, 1], F32, tag="misc")
        nc.sync.dma_start(out=qg, in_=cc_out.ap()[SR : SR + 1, 0:1])
        lt = pp.tile([1, 3], F32)
        nc.scalar.activation(lt[0:1, 0:1], qg[0:1, :], AF.Copy, bias=1.0, scale=-1.0 / (B * L))
        nc.scalar.activation(lt[0:1, 1:2], psm[:1, :1], AF.Copy, bias=1.0, scale=-1.0 / SR)
        nc.vector.tensor_add(lt[0:1, 2:3], lt[0:1, 0:1], lt[0:1, 1:2])
        nc.sync.dma_start(out=loss_o.ap(), in_=lt)

    nc.finalize()
    return nc


_NC = None


def _get_nc():
    global _NC
    if _NC is None:
        _NC = _build()
    return _NC


def _execute(query_features, support_features, trace=False):
    q = np.ascontiguousarray(query_features, dtype=np.float32).reshape(B, L, C)
    s = np.ascontiguousarray(support_features, dtype=np.float32).reshape(SR, C)
    asn_y = np.zeros((G, GCH * 128), dtype=np.float32)
    for r in range(GL):
        asn_y[r // L, r] = 1.0
    asn_x = np.zeros((BL, QCH * 128), dtype=np.float32)
    for r in range(QR):
        asn_x[r // L, r] = 1.0
    in_maps = [
        {
            "q_in": q[BL * c : BL * (c + 1)].reshape(QR, C),
            "s_in": s,
            "asn_y": asn_y,
            "asn_x": asn_x,
        }
        for c in range(NCORES)
    ]
    nc = _get_nc()
    return run_bass_kernel_spmd(nc, in_maps, list(range(NCORES)), trace=trace)


def kernel(query_features, support_features):
    res = _execute(query_features, support_features).results
    attn_q = np.concatenate([r["attn_q"] for r in res], axis=0)
    attn_s = np.concatenate([r["attn_s"] for r in res], axis=1)
    recon_q = np.concatenate(
        [r["recon_q"].reshape(BL, L, C) for r in res], axis=0
    )
    recon_s = res[0]["recon_s"].reshape(K, L, C)
    loss = res[0]["loss"][0]
    q_loss = np.float32(loss[0])
    s_loss = np.float32(loss[1])
    total = np.float32(loss[2])
    return (recon_q, recon_s, q_loss, s_loss, total, attn_q, attn_s)


# revision 8
# speedup vs baseline: 96.9532x; 96.9532x over previous
"""DualReconstruction Trainium2 kernel (8 NeuronCores, data-parallel over B).

Math (per reference):
  qn = l2norm(query), sn = l2norm(support)
  sim_q[b,k,i,j] = <qn[b,i], sn[k,j]>;  attn_q = softmax_j(sim_q)
  recon_q[b,i,c] = sum_{k,j} attn_q[b,k,i,j] support[k,j,c]
  sim_s = sim_q transposed;              attn_s = softmax over query dim
  recon_s[k,i,c] = sum_{b,j} attn_s[k,b,i,j] query[b,j,c]   (all-reduced over b)
  cosine losses over reconstructions.

Sharding: each of the 8 cores owns 4 of the 32 queries (B dim) and the full
support set.  Only recon_s (+ the q_loss scalar) needs an AllReduce.

|sim| <= 1 (unit vectors), so softmax skips the max-subtraction: exp is
computed directly and the row-sum comes for free via activation accum_out.
Matmuls run in float32r (e8m11) at full PE speed.
"""

import sys

sys.path.insert(0, "/opt/trn_rl_repo")

import numpy as np

import concourse.bacc as bacc
import concourse.mybir as mybir
import concourse.tile as tile
from concourse.bass_utils import run_bass_kernel_spmd
from concourse.masks import make_identity

F32 = mybir.dt.float32
F32R = mybir.dt.float32r
AF = mybir.ActivationFunctionType
AX = mybir.AxisListType.X

B, L, C, K = 32, 196, 384, 25
NCORES = 8
BL = B // NCORES          # 4 queries per core
QR = BL * L               # 784 stacked query rows
SR = K * L                # 4900 stacked support rows
G = 5                     # support images per group
NG = K // G               # 5 groups
GL = G * L                # 980 support rows per group
QCH = 7                   # ceil(784/128)
GCH = 8                   # ceil(980/128)
CCH = 3                   # 384/128
EPS = 1e-8

CNT_Q = [128] * 6 + [16]
CNT_G = [128] * 7 + [84]

# exp/rowsum sub-slices of the two 490-wide S1 PSUM passes:
# (col0, col1, accum_idx); k2 spans both passes (idx 2 + idx 3).
_S1_SLICES = {
    (0, 490): [(0, 196, 0), (196, 392, 1), (392, 490, 2)],
    (490, 980): [(490, 588, 3), (588, 784, 4), (784, 980, 5)],
}


def _pieces(lo, hi):
    """Intersect row range [lo,hi) with 128-row chunks -> (chunk, a, b)."""
    out = []
    for ch in range(lo // 128, (hi + 127) // 128):
        a = max(lo - 128 * ch, 0)
        b = min(hi - 128 * ch, 128)
        if a < b:
            out.append((ch, a, b))
    return out


def _build():
    nc = bacc.Bacc()
    q_in = nc.declare_dram_parameter("q_in", [QR, C], F32, isOutput=False)
    s_in = nc.declare_dram_parameter("s_in", [SR, C], F32, isOutput=False)
    # row->slab assignment matrices (host-built constants):
    # asn_y[k, r] = 1 if group row r belongs to support image k (r//196 == k)
    # asn_x[b, r] = 1 if query row r belongs to image b
    asn_y_in = nc.declare_dram_parameter("asn_y", [G, GCH * 128], F32, isOutput=False)
    asn_x_in = nc.declare_dram_parameter("asn_x", [BL, QCH * 128], F32, isOutput=False)
    attn_q_o = nc.declare_dram_parameter("attn_q", [BL, K, L, L], F32, isOutput=True)
    attn_s_o = nc.declare_dram_parameter("attn_s", [K, BL, L, L], F32, isOutput=True)
    recon_q_o = nc.declare_dram_parameter("recon_q", [QR, C], F32, isOutput=True)
    recon_s_o = nc.declare_dram_parameter("recon_s", [SR, C], F32, isOutput=True)
    loss_o = nc.declare_dram_parameter("loss", [1, 3], F32, isOutput=True)

    # per-group collective staging (separate tensors so the 5 all-reduces
    # pipeline independently), plus one row for the q_loss partial
    cc_in = [nc.dram_tensor(f"cc_in{g}", [GL, C], F32) for g in range(NG)]
    cc_out = [
        nc.dram_tensor(f"cc_out{g}", [GL, C], F32, addr_space="Shared")
        for g in range(NG)
    ]
    ql_in = nc.dram_tensor("ql_in", [1, C], F32)
    ql_out = nc.dram_tensor("ql_out", [1, C], F32, addr_space="Shared")
    rgroups = [list(range(NCORES))]

    with (
        tile.TileContext(nc) as tc,
        tc.tile_pool(name="pp", bufs=1) as pp,
        tc.tile_pool(name="gp", bufs=1) as gp,
        tc.tile_pool(name="scr", bufs=2) as scr,
        tc.tile_pool(name="ps_sim", bufs=4, space="PSUM") as ps_sim,
        tc.tile_pool(name="ps_bc", bufs=1, space="PSUM") as ps_bc,
        tc.tile_pool(name="ps_rc", bufs=2, space="PSUM") as ps_rc,
    ):
        # ---------- constants ----------
        ident = pp.tile([128, 128], F32)
        make_identity(nc, ident)
        ones_f = pp.tile([128, 1], F32)  # all-ones column (partition sums)
        nc.vector.memset(ones_f, 1.0)
        # row->slab assignment matrices (from host), used to build
        # per-chunk broadcast matrices on the PE
        asn_y = pp.tile([G, GCH * 128], F32R)
        nc.sync.dma_start(out=asn_y, in_=asn_y_in.ap().bitcast(F32R))
        asn_x = pp.tile([BL, QCH * 128], F32R)
        nc.sync.dma_start(out=asn_x, in_=asn_x_in.ap().bitcast(F32R))

        # ---------- load queries, normalize, transpose ----------
        q_r = pp.tile([128, QCH, C], F32R)
        for ch in range(QCH):
            cnt = CNT_Q[ch]
            nc.sync.dma_start(
                out=q_r[:cnt, ch, :],
                in_=q_in.ap()[128 * ch : 128 * ch + cnt, :].bitcast(F32R),
            )
        qss = pp.tile([128, QCH], F32)
        nc.vector.memset(qss, 1.0)
        for ch in range(QCH):
            cnt = CNT_Q[ch]
            sq = scr.tile([128, C], F32, tag="sq")
            nc.scalar.activation(
                sq[:cnt], q_r[:cnt, ch, :].bitcast(F32), AF.Square,
                accum_out=qss[:cnt, ch : ch + 1],
            )
        qnorm = pp.tile([128, QCH], F32)
        nc.scalar.activation(qnorm, qss, AF.Sqrt)
        nc.vector.tensor_scalar_max(qnorm, qnorm, EPS)
        inv_nq = pp.tile([128, QCH], F32)
        nc.vector.reciprocal(inv_nq, qnorm)
        qn = pp.tile([128, QCH, C], F32)
        for ch in range(QCH):
            cnt = CNT_Q[ch]
            nc.vector.tensor_scalar_mul(
                qn[:cnt, ch, :], q_r[:cnt, ch, :].bitcast(F32), inv_nq[:cnt, ch : ch + 1]
            )
        qnT = pp.tile([128, CCH, QR], F32R)
        for ch in range(QCH):
            cnt = CNT_Q[ch]
            for cc in range(CCH):
                ptr = ps_sim.tile([128, 490], F32, tag="sim")
                nc.tensor.transpose(
                    ptr[:128, :cnt], qn[:cnt, ch, 128 * cc : 128 * (cc + 1)],
                    ident[:cnt, :cnt],
                )
                nc.vector.tensor_copy(
                    qnT[:, cc, 128 * ch : 128 * ch + cnt], ptr[:128, :cnt].bitcast(F32R)
                )

        # ---------- support norms (group-chunk layout) ----------
        sss = pp.tile([128, NG * GCH], F32)
        nc.vector.memset(sss, 1.0)
        for g in range(NG):
            for j in range(GCH):
                cnt = CNT_G[j]
                r0 = GL * g + 128 * j
                st = scr.tile([128, C], F32, tag="schunk")
                nc.sync.dma_start(out=st[:cnt], in_=s_in.ap()[r0 : r0 + cnt, :])
                sq = scr.tile([128, C], F32, tag="sq")
                gc = g * GCH + j
                nc.scalar.activation(
                    sq[:cnt], st[:cnt], AF.Square, accum_out=sss[:cnt, gc : gc + 1]
                )
        snorm = pp.tile([128, NG * GCH], F32)
        nc.scalar.activation(snorm, sss, AF.Sqrt)
        nc.vector.tensor_scalar_max(snorm, snorm, EPS)
        inv_ns = pp.tile([128, NG * GCH], F32)
        nc.vector.reciprocal(inv_ns, snorm)

        rq_acc = pp.tile([128, QCH, C], F32)
        nc.vector.memset(rq_acc, 0.0)

        # ---------- main loop over support groups ----------
        for g in range(NG):
            k0 = G * g
            s_r = gp.tile([128, GCH, C], F32R, tag="s_r")
            for j in range(GCH):
                cnt = CNT_G[j]
                r0 = GL * g + 128 * j
                nc.scalar.dma_start(
                    out=s_r[:cnt, j, :],
                    in_=s_in.ap()[r0 : r0 + cnt, :].bitcast(F32R),
                )
            # normalized-support transpose for this group
            snT = gp.tile([128, CCH, GL], F32R, tag="snT")
            for j in range(GCH):
                cnt = CNT_G[j]
                gc = g * GCH + j
                snc = scr.tile([128, C], F32, tag="snc")
                nc.vector.tensor_scalar_mul(
                    snc[:cnt], s_r[:cnt, j, :].bitcast(F32), inv_ns[:cnt, gc : gc + 1]
                )
                for cc in range(CCH):
                    ptr = ps_sim.tile([128, 490], F32, tag="sim")
                    nc.tensor.transpose(
                        ptr[:128, :cnt], snc[:cnt, 128 * cc : 128 * (cc + 1)],
                        ident[:cnt, :cnt],
                    )
                    nc.vector.tensor_copy(
                        snT[:, cc, 128 * j : 128 * j + cnt], ptr[:128, :cnt].bitcast(F32R)
                    )

            # S1 = qn @ snT (rows: stacked queries), exp + per-k rowsums
            E1 = gp.tile([128, QCH, GL], F32, tag="E1")
            rsq_p = gp.tile([128, QCH, 6], F32, tag="rsq_p")
            nc.vector.memset(rsq_p, 1.0)
            for ch in range(QCH):
                cnt = CNT_Q[ch]
                for (n0, n1) in ((0, 490), (490, 980)):
                    pm = ps_sim.tile([128, 490], F32, tag="sim")
                    for cc in range(CCH):
                        nc.tensor.matmul(
                            pm[:cnt, : n1 - n0],
                            qnT[:, cc, 128 * ch : 128 * ch + cnt],
                            snT[:, cc, n0:n1],
                            start=(cc == 0), stop=(cc == CCH - 1),
                        )
                    for (a, b2, idx) in _S1_SLICES[(n0, n1)]:
                        nc.scalar.activation(
                            E1[:cnt, ch, a:b2], pm[:cnt, a - n0 : b2 - n0], AF.Exp,
                            accum_out=rsq_p[:cnt, ch, idx : idx + 1],
                        )
            rsq_inv = gp.tile([128, QCH, G], F32, tag="rsq_inv")
            nc.vector.tensor_copy(rsq_inv[:, :, 0], rsq_p[:, :, 0])
            nc.vector.tensor_copy(rsq_inv[:, :, 1], rsq_p[:, :, 1])
            nc.vector.tensor_add(rsq_inv[:, :, 2], rsq_p[:, :, 2], rsq_p[:, :, 3])
            nc.vector.tensor_copy(rsq_inv[:, :, 3], rsq_p[:, :, 4])
            nc.vector.tensor_copy(rsq_inv[:, :, 4], rsq_p[:, :, 5])
            nc.vector.reciprocal(rsq_inv, rsq_inv)

            # S2 = sn @ qnT (rows: this group's support), exp + per-b rowsums
            E2 = gp.tile([128, GCH, QR], F32, tag="E2")
            rss_s = gp.tile([128, GCH, BL], F32, tag="rss_s")
            nc.vector.memset(rss_s, 1.0)
            for j in range(GCH):
                cnt = CNT_G[j]
                for (n0, n1) in ((0, 392), (392, 784)):
                    pm = ps_sim.tile([128, 490], F32, tag="sim")
                    for cc in range(CCH):
                        nc.tensor.matmul(
                            pm[:cnt, : n1 - n0],
                            snT[:, cc, 128 * j : 128 * j + cnt],
                            qnT[:, cc, n0:n1],
                            start=(cc == 0), stop=(cc == CCH - 1),
                        )
                    for bi in range(2):
                        b = n0 // 196 + bi
                        a0 = 196 * b
                        nc.scalar.activation(
                            E2[:cnt, j, a0 : a0 + 196], pm[:cnt, a0 - n0 : a0 - n0 + 196],
                            AF.Exp, accum_out=rss_s[:cnt, j, b : b + 1],
                        )
            rss_inv = gp.tile([128, GCH, BL], F32, tag="rss_inv")
            nc.vector.reciprocal(rss_inv, rss_s)

            # transpose the inverse rowsums into rows (for broadcasts)
            irqT = gp.tile([G, QR], F32R, tag="irqT")
            for ch in range(QCH):
                cnt = CNT_Q[ch]
                ptr = ps_sim.tile([128, 490], F32, tag="sim")
                nc.tensor.transpose(
                    ptr[:G, :cnt], rsq_inv[:cnt, ch, :], ident[:cnt, :cnt]
                )
                nc.vector.tensor_copy(
                    irqT[:, 128 * ch : 128 * ch + cnt], ptr[:G, :cnt].bitcast(F32R)
                )
            irsT = gp.tile([BL, GL], F32R, tag="irsT")
            for j in range(GCH):
                cnt = CNT_G[j]
                ptr = ps_sim.tile([128, 490], F32, tag="sim")
                nc.tensor.transpose(
                    ptr[:BL, :cnt], rss_inv[:cnt, j, :], ident[:cnt, :cnt]
                )
                nc.vector.tensor_copy(
                    irsT[:, 128 * j : 128 * j + cnt], ptr[:BL, :cnt].bitcast(F32R)
                )

            # Y = attn_q^T = E2 * inv_rsq[bq] (row broadcast via asn matmul)
            Y = gp.tile([128, GCH, QR], F32R, tag="Y")
            for j in range(GCH):
                cnt = CNT_G[j]
                pb = ps_bc.tile([128, 2, 512], F32, tag="bc")
                for h in range(2):
                    nc.tensor.matmul(
                        pb[:, h, :392],
                        asn_y[:, 128 * j : 128 * (j + 1)],
                        irqT[:, 392 * h : 392 * (h + 1)],
                        start=True, stop=True,
                    )
                    nc.vector.tensor_mul(
                        Y[:cnt, j, 392 * h : 392 * (h + 1)],
                        E2[:cnt, j, 392 * h : 392 * (h + 1)].bitcast(F32R),
                        pb[:cnt, h, :392].bitcast(F32R),
                    )
            # X = attn_s^T = E1 * inv_rss[ks] (row broadcast via asn matmul)
            X = gp.tile([128, QCH, GL], F32R, tag="X")
            for ch in range(QCH):
                cnt = CNT_Q[ch]
                pb = ps_bc.tile([128, 2, 512], F32, tag="bc")
                for h in range(2):
                    nc.tensor.matmul(
                        pb[:, h, :490],
                        asn_x[:, 128 * ch : 128 * (ch + 1)],
                        irsT[:, 490 * h : 490 * (h + 1)],
                        start=True, stop=True,
                    )
                    nc.vector.tensor_mul(
                        X[:cnt, ch, 490 * h : 490 * (h + 1)],
                        E1[:cnt, ch, 490 * h : 490 * (h + 1)].bitcast(F32R),
                        pb[:cnt, h, :490].bitcast(F32R),
                    )

            # attn_q: scale E1 rows in place, then DMA out
            for ch in range(QCH):
                cnt = CNT_Q[ch]
                e1v = E1[:cnt, ch, :].rearrange("p (k s) -> p k s", k=G)
                nc.vector.tensor_mul(
                    e1v, e1v,
                    rsq_inv[:cnt, ch, :].unsqueeze(2).to_broadcast([cnt, G, L]),
                )
            for b in range(BL):
                for kl in range(G):
                    for (ch, a, b2) in _pieces(196 * b, 196 * (b + 1)):
                        i0 = 128 * ch + a - 196 * b
                        nc.sync.dma_start(
                            out=attn_q_o.ap()[b, k0 + kl, i0 : i0 + (b2 - a), :],
                            in_=E1[a:b2, ch, 196 * kl : 196 * (kl + 1)],
                        )
            # attn_s: scale E2 rows in place, then DMA out
            for j in range(GCH):
                cnt = CNT_G[j]
                e2v = E2[:cnt, j, :].rearrange("p (b s) -> p b s", b=BL)
                nc.vector.tensor_mul(
                    e2v, e2v,
                    rss_inv[:cnt, j, :].unsqueeze(2).to_broadcast([cnt, BL, L]),
                )
            for kl in range(G):
                for b in range(BL):
                    for (j, a, b2) in _pieces(196 * kl, 196 * (kl + 1)):
                        i0 = 128 * j + a - 196 * kl
                        nc.scalar.dma_start(
                            out=attn_s_o.ap()[k0 + kl, b, i0 : i0 + (b2 - a), :],
                            in_=E2[a:b2, j, 196 * b : 196 * (b + 1)],
                        )

            # recon_q partial: accumulate Y^T @ support over this group
            for ch in range(QCH):
                cnt = CNT_Q[ch]
                pr = ps_rc.tile([128, C], F32, tag="rc")
                for j in range(GCH):
                    cj = CNT_G[j]
                    nc.tensor.matmul(
                        pr[:cnt, :],
                        Y[:cj, j, 128 * ch : 128 * ch + cnt],
                        s_r[:cj, j, :],
                        start=(j == 0), stop=(j == GCH - 1),
                    )
                nc.vector.tensor_add(rq_acc[:cnt, ch, :], rq_acc[:cnt, ch, :], pr[:cnt, :])

            # recon_s rows (final for this group, pre-allreduce)
            for j in range(GCH):
                cnt = CNT_G[j]
                pr = ps_rc.tile([128, C], F32, tag="rc")
                for ch in range(QCH):
                    cq = CNT_Q[ch]
                    nc.tensor.matmul(
                        pr[:cnt, :],
                        X[:cq, ch, 128 * j : 128 * j + cnt],
                        q_r[:cq, ch, :],
                        start=(ch == 0), stop=(ch == QCH - 1),
                    )
                stg = scr.tile([128, C], F32, tag="rs_stage")
                nc.scalar.copy(stg[:cnt], pr[:cnt, :])
                nc.sync.dma_start(
                    out=cc_in[g].ap()[128 * j : 128 * j + cnt, :], in_=stg[:cnt]
                )
            # all-reduce this group's recon_s partial now (overlaps next groups)
            nc.gpsimd.collective_compute(
                "AllReduce", mybir.AluOpType.add, replica_groups=rgroups,
                ins=[cc_in[g].ap()], outs=[cc_out[g].ap()],
            )
            # global recon_s rows straight out via DRAM->DRAM DMA
            nc.scalar.dma_start(
                out=recon_s_o.ap()[GL * g : GL * (g + 1), :], in_=cc_out[g].ap()
            )

        # ---------- epilogue: recon_q out + q_loss partial ----------
        rqss = pp.tile([128, QCH], F32)
        nc.vector.memset(rqss, 1.0)
        dots_q = pp.tile([128, QCH], F32)
        nc.vector.memset(dots_q, 0.0)
        for ch in range(QCH):
            cnt = CNT_Q[ch]
            nc.sync.dma_start(
                out=recon_q_o.ap()[128 * ch : 128 * ch + cnt, :], in_=rq_acc[:cnt, ch, :]
            )
            sq = scr.tile([128, C], F32, tag="sq")
            nc.scalar.activation(
                sq[:cnt], rq_acc[:cnt, ch, :], AF.Square, accum_out=rqss[:cnt, ch : ch + 1]
            )
            t = scr.tile([128, C], F32, tag="dot")
            nc.vector.tensor_mul(t[:cnt], rq_acc[:cnt, ch, :], qn[:cnt, ch, :])
            nc.vector.reduce_sum(dots_q[:cnt, ch : ch + 1], t[:cnt], axis=AX)
        rqn = pp.tile([128, QCH], F32)
        nc.scalar.activation(rqn, rqss, AF.Sqrt)
        nc.vector.tensor_scalar_max(rqn, rqn, EPS)
        inv_rqn = pp.tile([128, QCH], F32)
        nc.vector.reciprocal(inv_rqn, rqn)
        nc.vector.tensor_mul(dots_q, dots_q, inv_rqn)
        dcs = pp.tile([128, 1], F32)
        nc.vector.reduce_sum(dcs, dots_q, axis=AX)
        pq = ps_sim.tile([128, 490], F32, tag="sim")
        nc.tensor.matmul(pq[:1, :1], ones_f[:, :1], dcs[:, :1], start=True, stop=True)
        zrow = pp.tile([1, C], F32)
        nc.vector.memset(zrow, 0.0)
        nc.scalar.copy(zrow[0:1, 0:1], pq[:1, :1])
        nc.sync.dma_start(out=ql_in.ap(), in_=zrow)
        nc.gpsimd.collective_compute(
            "AllReduce", mybir.AluOpType.add, replica_groups=rgroups,
            ins=[ql_in.ap()], outs=[ql_out.ap()],
        )

        # ---------- final: recon_s out + s_loss + losses ----------
        dots_s = pp.tile([128, NG * GCH], F32)
        nc.vector.memset(dots_s, 0.0)
        rsss = pp.tile([128, NG * GCH], F32)
        nc.vector.memset(rsss, 1.0)
        for g in range(NG):
            for j in range(GCH):
                cnt = CNT_G[j]
                gc = g * GCH + j
                r0 = GL * g + 128 * j
                rsg = scr.tile([128, C], F32, tag="rsg")
                nc.sync.dma_start(
                    out=rsg[:cnt], in_=cc_out[g].ap()[128 * j : 128 * j + cnt, :]
                )
                sq = scr.tile([128, C], F32, tag="sq")
                nc.scalar.activation(
                    sq[:cnt], rsg[:cnt], AF.Square, accum_out=rsss[:cnt, gc : gc + 1]
                )
                st = scr.tile([128, C], F32, tag="schunk")
                nc.sync.dma_start(out=st[:cnt], in_=s_in.ap()[r0 : r0 + cnt, :])
                t = scr.tile([128, C], F32, tag="dot")
                nc.vector.tensor_mul(t[:cnt], rsg[:cnt], st[:cnt])
                nc.vector.reduce_sum(dots_s[:cnt, gc : gc + 1], t[:cnt], axis=AX)
        rsn = pp.tile([128, NG * GCH], F32)
        nc.scalar.activation(rsn, rsss, AF.Sqrt)
        nc.vector.tensor_scalar_max(rsn, rsn, EPS)
        inv_rsn = pp.tile([128, NG * GCH], F32)
        nc.vector.reciprocal(inv_rsn, rsn)
        nc.vector.tensor_mul(dots_s, dots_s, inv_rsn)
        nc.vector.tensor_mul(dots_s, dots_s, inv_ns)
        scs = pp.tile([128, 1], F32)
        nc.vector.reduce_sum(scs, dots_s, axis=AX)
        psm = ps_sim.tile([128, 490], F32, tag="sim")
        nc.tensor.matmul(psm[:1, :1], ones_f[:, :1], scs[:, :1], start=True, stop=True)

        qg = scr.tile([1, 1], F32, tag="misc")
        nc.sync.dma_start(out=qg, in_=ql_out.ap()[0:1, 0:1])
        lt = pp.tile([1, 3], F32)
        nc.scalar.activation(lt[0:1, 0:1], qg[0:1, :], AF.Copy, bias=1.0, scale=-1.0 / (B * L))
        nc.scalar.activation(lt[0:1, 1:2], psm[:1, :1], AF.Copy, bias=1.0, scale=-1.0 / SR)
        nc.vector.tensor_add(lt[0:1, 2:3], lt[0:1, 0:1], lt[0:1, 1:2])
        nc.sync.dma_start(out=loss_o.ap(), in_=lt)

    nc.finalize()
    return nc


_NC = None


def _get_nc():
    global _NC
    if _NC is None:
        _NC = _build()
    return _NC


def _execute(query_features, support_features, trace=False):
    q = np.ascontiguousarray(query_features, dtype=np.float32).reshape(B, L, C)
    s = np.ascontiguousarray(support_features, dtype=np.float32).reshape(SR, C)
    asn_y = np.zeros((G, GCH * 128), dtype=np.float32)
    for r in range(GL):
        asn_y[r // L, r] = 1.0
    asn_x = np.zeros((BL, QCH * 128), dtype=np.float32)
    for r in range(QR):
        asn_x[r // L, r] = 1.0
    in_maps = [
        {
            "q_in": q[BL * c : BL * (c + 1)].reshape(QR, C),
            "s_in": s,
            "asn_y": asn_y,
            "asn_x": asn_x,
        }
        for c in range(NCORES)
    ]
    nc = _get_nc()
    return run_bass_kernel_spmd(nc, in_maps, list(range(NCORES)), trace=trace)


def kernel(query_features, support_features):
    res = _execute(query_features, support_features).results
    attn_q = np.concatenate([r["attn_q"] for r in res], axis=0)
    attn_s = np.concatenate([r["attn_s"] for r in res], axis=1)
    recon_q = np.concatenate(
        [r["recon_q"].reshape(BL, L, C) for r in res], axis=0
    )
    recon_s = res[0]["recon_s"].reshape(K, L, C)
    loss = res[0]["loss"][0]
    q_loss = np.float32(loss[0])
    s_loss = np.float32(loss[1])
    total = np.float32(loss[2])
    return (recon_q, recon_s, q_loss, s_loss, total, attn_q, attn_s)
